# revision 2
# baseline (speedup 1.0000x reference)
"""Windowed cross-attention with relative position encodings, data-parallel
over batch across 8 NeuronCores.

Sharding (per spec hint): B=32 is split 4-per-core across the 8 cores;
the small q/kv/proj weights and the 169x1152 RPE table are replicated.
Windows are independent so attention needs no cross-device communication.

All einsums are rewritten as plain batched matmuls (lax.dot_general with
leading batch dims) so XLA-neuron lowers them to TensorE batched GEMMs
instead of gather loops.  The static RPE gather (169 -> [49,49] table) is
folded on the host into dense per-(h,i,c,j) tables, and matmul operands are
cast to bf16 (f32 accumulation) to double PE throughput.
"""

import numpy as np

import jax
import jax.numpy as jnp

WS = 7
NH = 12
DIM = 384
HD = DIM // NH
L = WS * WS
SCALE = HD ** (-0.5)
N_CORES = 8

BF = jnp.bfloat16


def _relative_position_index() -> np.ndarray:
    coords = np.stack(np.meshgrid(np.arange(WS), np.arange(WS), indexing="ij"))
    flat = coords.reshape(2, -1)
    rel = flat[:, :, None] - flat[:, None, :]
    rel = rel.transpose(1, 2, 0).copy()
    rel[:, :, 0] += WS - 1
    rel[:, :, 1] += WS - 1
    rel[:, :, 0] *= 2 * WS - 1
    return rel.sum(-1)  # [L, L] int


_RPI = _relative_position_index()


def _partition(t, b):
    # [b, 56, 56, DIM] -> [b*64, L, NH*HD] window-major tokens
    nh = 56 // WS
    t = t.reshape(b, nh, WS, nh, WS, DIM)
    t = t.transpose(0, 1, 3, 2, 4, 5)  # b, wi, wj, ih, iw, d
    return t.reshape(b * nh * nh, L, DIM)


def _unpartition(t, b):
    nh = 56 // WS
    t = t.reshape(b, nh, nh, WS, WS, DIM)
    t = t.transpose(0, 1, 3, 2, 4, 5)
    return t.reshape(b, 56, 56, DIM)


def _bmm(a, b):
    # batched matmul over arbitrary leading dims, f32 accumulation
    return jax.lax.dot_general(
        a, b,
        dimension_numbers=(((a.ndim - 1,), (b.ndim - 2,)),
                           (tuple(range(a.ndim - 2)), tuple(range(b.ndim - 2)))),
        preferred_element_type=jnp.float32,
    )


def _core_fn(x, context, q_w, q_b, kv_w, kv_b, proj_w, proj_b,
             k_rpe_t, q_rpe_t, v_rpe_t):
    """Per-core compute.  x, context: [b, 56, 56, DIM] bf16.

    k_rpe_t: [NH, L, HD, L]  (h, i, c, j)   -- already SCALE-free
    q_rpe_t: [NH, L, HD, L]  (h, j, c, i)   -- already * SCALE
    v_rpe_t: [NH, L, L, HD]  (h, i, j, c)
    """
    b = x.shape[0]
    bw = b * 64

    # --- projections (big dense GEMMs, bf16 x bf16 -> f32) ---
    xw = _partition(x, b)                      # [bw, L, DIM]
    cw = _partition(context, b)                # [bw, L, DIM]

    q = (_bmm(xw.reshape(bw * L, DIM), q_w) + q_b).astype(BF)      # [bw*L, DIM]
    kv = (_bmm(cw.reshape(bw * L, DIM), kv_w) + kv_b).astype(BF)   # [bw*L, 2*DIM]
    k = kv[:, :DIM]
    v = kv[:, DIM:]

    # [bw, NH, L, HD]
    q = q.reshape(bw, L, NH, HD).transpose(0, 2, 1, 3) * jnp.asarray(SCALE, BF)
    k = k.reshape(bw, L, NH, HD).transpose(0, 2, 1, 3)
    v = v.reshape(bw, L, NH, HD).transpose(0, 2, 1, 3)

    # --- attention logits ---
    # qk: [bw, NH, L, L]
    qk = _bmm(q, k.transpose(0, 1, 3, 2))

    # qr[b,h,i,j] = sum_c q[b,h,i,c] * k_rpe[h,i,c,j]
    #   batch dims (h, i): q' [NH, L, bw, HD] @ k_rpe_t [NH, L, HD, L]
    qh = q.transpose(1, 2, 0, 3)               # [NH, L, bw, HD]
    qr = _bmm(qh, k_rpe_t)                     # [NH, L, bw, L] (h, i, b, j)
    qr = qr.transpose(2, 0, 1, 3)              # [bw, NH, L, L]

    # kr[b,h,i,j] = sum_c k[b,h,j,c] * q_rpe[h,j,c,i]
    kh = k.transpose(1, 2, 0, 3)               # [NH, L(j), bw, HD]
    kr = _bmm(kh, q_rpe_t)                     # [NH, L(j), bw, L(i)]
    kr = kr.transpose(2, 0, 3, 1)              # [bw, NH, L(i), L(j)]

    s = qk + qr + kr

    # --- softmax over j (logits are small: skip max-subtraction) ---
    p = jnp.exp(s)
    p = p / jnp.sum(p, axis=-1, keepdims=True)
    p = p.astype(BF)

    # --- values ---
    o = _bmm(p, v)                             # [bw, NH, L, HD]

    # o2[b,h,i,c] = sum_j p[b,h,i,j] * v_rpe[h,i,j,c]
    ph = p.transpose(1, 2, 0, 3)               # [NH, L(i), bw, L(j)]
    o2 = _bmm(ph, v_rpe_t)                     # [NH, L, bw, HD]
    o2 = o2.transpose(2, 0, 1, 3)              # [bw, NH, L, HD]

    o = (o + o2).transpose(0, 2, 1, 3).reshape(bw * L, DIM).astype(BF)

    # --- output projection ---
    y = _bmm(o, proj_w) + proj_b               # [bw*L, DIM] f32
    return _unpartition(y.reshape(bw, L, DIM), b)


_PMAP = None


def _get_pmap():
    global _PMAP
    if _PMAP is None:
        _PMAP = jax.pmap(_core_fn, devices=jax.devices()[:N_CORES])
    return _PMAP


def _tile8(a):
    a = np.asarray(a)
    return np.broadcast_to(a, (N_CORES,) + a.shape)


def _prep_consts(rpe_table, q_w, q_b, kv_w, kv_b, proj_w, proj_b):
    # host-side fold of the static gather: [169, 1152] -> dense tables
    rpe = np.asarray(rpe_table)[_RPI.reshape(-1)].reshape(L, L, NH, 3 * HD)
    q_rpe, k_rpe, v_rpe = np.split(rpe, 3, axis=-1)   # [L(i), L(j), NH, HD]
    # k_rpe_t[h, i, c, j]
    k_rpe_t = k_rpe.transpose(2, 0, 3, 1).astype(ml_bf16())
    # q_rpe_t[h, j, c, i] (with SCALE folded in)
    q_rpe_t = (q_rpe * SCALE).transpose(2, 1, 3, 0).astype(ml_bf16())
    # v_rpe_t[h, i, j, c]
    v_rpe_t = v_rpe.transpose(2, 0, 1, 3).astype(ml_bf16())
    return dict(
        q_w=np.asarray(q_w).astype(ml_bf16()),
        q_b=np.asarray(q_b, np.float32),
        kv_w=np.asarray(kv_w).astype(ml_bf16()),
        kv_b=np.asarray(kv_b, np.float32),
        proj_w=np.asarray(proj_w).astype(ml_bf16()),
        proj_b=np.asarray(proj_b, np.float32),
        k_rpe_t=k_rpe_t, q_rpe_t=q_rpe_t, v_rpe_t=v_rpe_t,
    )


def ml_bf16():
    import ml_dtypes
    return ml_dtypes.bfloat16


def kernel(x, context, rpe_table, q_w, q_b, kv_w, kv_b, proj_w, proj_b):
    x = np.asarray(x)
    context = np.asarray(context)
    B = x.shape[0]
    per = B // N_CORES

    consts = _prep_consts(rpe_table, q_w, q_b, kv_w, kv_b, proj_w, proj_b)

    xs = x.reshape(N_CORES, per, 56, 56, DIM).astype(ml_bf16())
    cs = context.reshape(N_CORES, per, 56, 56, DIM).astype(ml_bf16())

    out = _get_pmap()(
        xs, cs,
        _tile8(consts["q_w"]), _tile8(consts["q_b"]),
        _tile8(consts["kv_w"]), _tile8(consts["kv_b"]),
        _tile8(consts["proj_w"]), _tile8(consts["proj_b"]),
        _tile8(consts["k_rpe_t"]), _tile8(consts["q_rpe_t"]),
        _tile8(consts["v_rpe_t"]),
    )
    out = np.asarray(out).reshape(B, 56, 56, DIM)
    return out.astype(np.float32)


# revision 4
# speedup vs baseline: 1.3409x; 1.3409x over previous
"""Windowed cross-attention with relative position encodings, data-parallel
over batch across 8 NeuronCores.

Sharding (per spec hint): B=32 is split 4-per-core across the 8 cores;
the small q/kv/proj weights and the 169x1152 RPE table are replicated.
Windows are independent so attention needs no cross-device communication.

All einsums are rewritten as plain batched matmuls (lax.dot_general with
leading batch dims) so XLA-neuron lowers them to TensorE batched GEMMs
instead of gather loops.  The static RPE gather (169 -> [49,49] table) is
folded on the host into dense per-(h,i,c,j) tables, and matmul operands are
cast to bf16 (f32 accumulation) to double PE throughput.
"""

import numpy as np

import jax
import jax.numpy as jnp

WS = 7
NH = 12
DIM = 384
HD = DIM // NH
L = WS * WS
SCALE = HD ** (-0.5)
N_CORES = 8

BF = jnp.bfloat16


def _relative_position_index() -> np.ndarray:
    coords = np.stack(np.meshgrid(np.arange(WS), np.arange(WS), indexing="ij"))
    flat = coords.reshape(2, -1)
    rel = flat[:, :, None] - flat[:, None, :]
    rel = rel.transpose(1, 2, 0).copy()
    rel[:, :, 0] += WS - 1
    rel[:, :, 1] += WS - 1
    rel[:, :, 0] *= 2 * WS - 1
    return rel.sum(-1)  # [L, L] int


_RPI = _relative_position_index()


def _partition(t, b):
    # [b, 56, 56, DIM] -> [b*64, L, NH*HD] window-major tokens
    nh = 56 // WS
    t = t.reshape(b, nh, WS, nh, WS, DIM)
    t = t.transpose(0, 1, 3, 2, 4, 5)  # b, wi, wj, ih, iw, d
    return t.reshape(b * nh * nh, L, DIM)


def _unpartition(t, b):
    nh = 56 // WS
    t = t.reshape(b, nh, nh, WS, WS, DIM)
    t = t.transpose(0, 1, 3, 2, 4, 5)
    return t.reshape(b, 56, 56, DIM)


def _bmm(a, b, out_dtype=jnp.float32):
    # batched matmul over arbitrary leading dims; PSUM accumulates f32,
    # out_dtype only controls the copy-out precision
    return jax.lax.dot_general(
        a, b,
        dimension_numbers=(((a.ndim - 1,), (b.ndim - 2,)),
                           (tuple(range(a.ndim - 2)), tuple(range(b.ndim - 2)))),
        preferred_element_type=out_dtype,
    )


def _core_fn(x, context, q_w, q_b, kv_w, kv_b, proj_w, proj_b,
             k_rpe_t, q_rpe_t, v_rpe_t):
    """Per-core compute.  x, context: [b, 56, 56, DIM] bf16.

    k_rpe_t: [NH, L, HD, L]  (h, i, c, j)   -- already SCALE-free
    q_rpe_t: [NH, L, HD, L]  (h, j, c, i)   -- already * SCALE
    v_rpe_t: [NH, L, L, HD]  (h, i, j, c)
    """
    b = x.shape[0]
    bw = b * 64

    # --- projections (big dense GEMMs, bf16 x bf16 -> f32) ---
    xw = _partition(x, b)                      # [bw, L, DIM]
    cw = _partition(context, b)                # [bw, L, DIM]

    q = (_bmm(xw.reshape(bw * L, DIM), q_w) + q_b).astype(BF)      # [bw*L, DIM]
    kv = (_bmm(cw.reshape(bw * L, DIM), kv_w) + kv_b).astype(BF)   # [bw*L, 2*DIM]
    k = kv[:, :DIM]
    v = kv[:, DIM:]

    # [bw, NH, L, HD]
    q = q.reshape(bw, L, NH, HD).transpose(0, 2, 1, 3) * jnp.asarray(SCALE, BF)
    k = k.reshape(bw, L, NH, HD).transpose(0, 2, 1, 3)
    v = v.reshape(bw, L, NH, HD).transpose(0, 2, 1, 3)

    # --- attention logits (bf16 copy-out; PSUM still accumulates f32) ---
    # qk: [bw, NH, L, L]
    qk = _bmm(q, k.transpose(0, 1, 3, 2), BF)

    # qr[b,h,i,j] = sum_c q[b,h,i,c] * k_rpe[h,i,c,j]
    #   batch dims (h, i): q' [NH, L, bw, HD] @ k_rpe_t [NH, L, HD, L]
    qh = q.transpose(1, 2, 0, 3)               # [NH, L, bw, HD]
    qr = _bmm(qh, k_rpe_t, BF)                 # [NH, L, bw, L] (h, i, b, j)
    qr = qr.transpose(2, 0, 1, 3)              # [bw, NH, L, L]

    # kr[b,h,i,j] = sum_c k[b,h,j,c] * q_rpe[h,j,c,i]
    kh = k.transpose(1, 2, 0, 3)               # [NH, L(j), bw, HD]
    kr = _bmm(kh, q_rpe_t, BF)                 # [NH, L(j), bw, L(i)]
    kr = kr.transpose(2, 0, 3, 1)              # [bw, NH, L(i), L(j)]

    s = qk + qr + kr                           # bf16

    # --- softmax over j (logits are small: skip max-subtraction) ---
    p = jnp.exp(s)                             # bf16
    z = jnp.sum(p, axis=-1, keepdims=True, dtype=jnp.float32)
    p = (p / z).astype(BF)

    # --- values ---
    o = _bmm(p, v, BF)                         # [bw, NH, L, HD]

    # o2[b,h,i,c] = sum_j p[b,h,i,j] * v_rpe[h,i,j,c]
    ph = p.transpose(1, 2, 0, 3)               # [NH, L(i), bw, L(j)]
    o2 = _bmm(ph, v_rpe_t, BF)                 # [NH, L, bw, HD]
    o2 = o2.transpose(2, 0, 1, 3)              # [bw, NH, L, HD]

    o = (o + o2).transpose(0, 2, 1, 3).reshape(bw * L, DIM)

    # --- output projection ---
    y = _bmm(o, proj_w) + proj_b               # [bw*L, DIM] f32
    return _unpartition(y.reshape(bw, L, DIM), b)


_PMAP = None


def _get_pmap():
    global _PMAP
    if _PMAP is None:
        _PMAP = jax.pmap(_core_fn, devices=jax.devices()[:N_CORES])
    return _PMAP


def _tile8(a):
    a = np.asarray(a)
    return np.broadcast_to(a, (N_CORES,) + a.shape)


def _prep_consts(rpe_table, q_w, q_b, kv_w, kv_b, proj_w, proj_b):
    # host-side fold of the static gather: [169, 1152] -> dense tables
    rpe = np.asarray(rpe_table)[_RPI.reshape(-1)].reshape(L, L, NH, 3 * HD)
    q_rpe, k_rpe, v_rpe = np.split(rpe, 3, axis=-1)   # [L(i), L(j), NH, HD]
    # k_rpe_t[h, i, c, j]
    k_rpe_t = k_rpe.transpose(2, 0, 3, 1).astype(ml_bf16())
    # q_rpe_t[h, j, c, i] (with SCALE folded in)
    q_rpe_t = (q_rpe * SCALE).transpose(2, 1, 3, 0).astype(ml_bf16())
    # v_rpe_t[h, i, j, c]
    v_rpe_t = v_rpe.transpose(2, 0, 1, 3).astype(ml_bf16())
    return dict(
        q_w=np.asarray(q_w).astype(ml_bf16()),
        q_b=np.asarray(q_b, np.float32),
        kv_w=np.asarray(kv_w).astype(ml_bf16()),
        kv_b=np.asarray(kv_b, np.float32),
        proj_w=np.asarray(proj_w).astype(ml_bf16()),
        proj_b=np.asarray(proj_b, np.float32),
        k_rpe_t=k_rpe_t, q_rpe_t=q_rpe_t, v_rpe_t=v_rpe_t,
    )


def ml_bf16():
    import ml_dtypes
    return ml_dtypes.bfloat16


def kernel(x, context, rpe_table, q_w, q_b, kv_w, kv_b, proj_w, proj_b):
    x = np.asarray(x)
    context = np.asarray(context)
    B = x.shape[0]
    per = B // N_CORES

    consts = _prep_consts(rpe_table, q_w, q_b, kv_w, kv_b, proj_w, proj_b)

    xs = x.reshape(N_CORES, per, 56, 56, DIM).astype(ml_bf16())
    cs = context.reshape(N_CORES, per, 56, 56, DIM).astype(ml_bf16())

    out = _get_pmap()(
        xs, cs,
        _tile8(consts["q_w"]), _tile8(consts["q_b"]),
        _tile8(consts["kv_w"]), _tile8(consts["kv_b"]),
        _tile8(consts["proj_w"]), _tile8(consts["proj_b"]),
        _tile8(consts["k_rpe_t"]), _tile8(consts["q_rpe_t"]),
        _tile8(consts["v_rpe_t"]),
    )
    out = np.asarray(out).reshape(B, 56, 56, DIM)
    return out.astype(np.float32)
